# revision 9
# baseline (speedup 1.0000x reference)
"""Trainium2 Bass kernel: weighted sliding-window min (STL 'Always' robustness).

out[n, w] = min_k( input[n, 4*w + k] * And_weight[0, k] ),  k in [0, 16)

Strategy (8 NeuronCores, data-parallel over batch N=1024 -> 128 rows/core):
  - Host: cast input to bf16 and deinterleave each row into 4 phase planes
    P_j[b] = x[4b + j]; pre-tile along the block axis with a 3-block halo.
  - Device: the 16 products p_{o,j} = P_j * c[4o+j] via tensor_scalar
    (bf16 4x mode on DVE, share on ScalarE), then a 4-level tensor_tensor
    min tree (bf16 2x on DVE, share on GpSimd) with shifts folded into
    access-pattern offsets (even shifts keep 4B alignment / 2x packing).
  - out[w] = min_o m_o[w+o] where m_o[b] = min_j P_j[b]*c[4o+j].
"""

import numpy as np

# Problem geometry (hardcoded; harness calls kernel() with these shapes)
N, L = 1024, 8192
K, S = 16, 4
W = (L - K) // S + 1          # 2045 output windows per row
NCORES = 8
ROWS = N // NCORES            # 128 rows per core == SBUF partitions
B = L // S                    # 2048 blocks of 4 per row

# Column tiling over the block axis
NT = 2                        # number of column tiles
BT = 1024                     # output windows per tile
TW = BT + 3                   # tile width in blocks (3-block halo)

_COMPILED = {}


def _build_bass():
    import concourse.bacc as bacc
    import concourse.mybir as mybir
    from concourse.tile import TileContext

    BF16 = mybir.dt.bfloat16
    F32 = mybir.dt.float32
    MIN = mybir.AluOpType.min

    nc = bacc.Bacc()
    x = nc.dram_tensor("x", [ROWS, NT, 4, TW], BF16, kind="ExternalInput")
    w = nc.dram_tensor("w", [ROWS, 16], F32, kind="ExternalInput")
    out = nc.dram_tensor("out", [ROWS, W], F32, kind="ExternalOutput")

    # slot(o, j): plane ordering that keeps every min-tree level a dense
    # step-1 access pattern:
    #   Q = [q0A q1A q0B q1B | q2A q3A q2B q3B]
    #   U = [uA vA uB vB],  R = [r0 r1]
    def slot(o, j):
        return 4 * (o // 2) + 2 * (j // 2) + (o % 2)

    with TileContext(nc) as tc:
        with (
            tc.tile_pool(name="wp", bufs=1) as wp,
            tc.tile_pool(name="xin", bufs=2) as xin,
            tc.tile_pool(name="pa", bufs=2) as pa,
            tc.tile_pool(name="pb", bufs=2) as pb,
            tc.tile_pool(name="qq", bufs=2) as qq,
            tc.tile_pool(name="uu", bufs=2) as uu,
            tc.tile_pool(name="rr", bufs=2) as rr,
            tc.tile_pool(name="oo", bufs=2) as oo,
        ):
            w_sb = wp.tile([ROWS, 16], F32)
            nc.sync.dma_start(out=w_sb[:, :], in_=w[:, :])
            # Ratio weights for the ScalarE product path. ACT instructions
            # can carry only ONE HW sync-wait, so every ACT op must depend on
            # a single semaphore (DVE's): ACT computes p_{o,j} for o in {2,3}
            # from DVE's p_{o-2,j} via p * (c[4o+j] / c[4(o-2)+j]).
            winv = wp.tile([ROWS, 8], F32)
            nc.vector.reciprocal(out=winv[:, :], in_=w_sb[:, 0:8])
            ratio = wp.tile([ROWS, 8], F32)
            nc.vector.tensor_tensor(
                out=ratio[:, :],
                in0=w_sb[:, 8:16],
                in1=winv[:, :],
                op=mybir.AluOpType.mult,
            )

            for t in range(NT):
                wbase = BT * t
                wcnt = min(BT, W - wbase)

                xt = xin.tile([ROWS, 4, TW], BF16)
                nc.sync.dma_start(out=xt[:, :, :], in_=x[:, t, :, :])

                A = pa.tile([ROWS, 8, TW], BF16)
                Bb = pb.tile([ROWS, 8, TW], BF16)
                # 16 products p_{o,j} = P_j * c[4o+j].
                # j even -> A buf, j odd -> B buf (L1 pairs (j=0,1) and (j=2,3)).
                # DVE: o in {0,1} direct from xt (slots 0..3, bf16 4x mode).
                for o in range(2):
                    for j in range(4):
                        dst = A if (j % 2 == 0) else Bb
                        s = slot(o, j)
                        nc.vector.tensor_scalar_mul(
                            out=dst[:, s, :],
                            in0=xt[:, j, :],
                            scalar1=w_sb[:, 4 * o + j : 4 * o + j + 1],
                        )
                # ACT: o in {2,3} via ratio-scaling DVE's planes (slots 4..7).
                for o in range(2, 4):
                    for j in range(4):
                        dst = A if (j % 2 == 0) else Bb
                        s = slot(o, j)
                        nc.scalar.mul(
                            out=dst[:, s, :],
                            in_=dst[:, s - 4, :],
                            mul=ratio[:, 4 * (o - 2) + j : 4 * (o - 2) + j + 1],
                        )

                # L1: q = min(p_{o, j even}, p_{o, j odd})  (8 planes)
                Q = qq.tile([ROWS, 8, TW], BF16)
                nc.vector.tensor_tensor(
                    out=Q[:, :, :], in0=A[:, :, :], in1=Bb[:, :, :], op=MIN
                )

                # L2: fold the o -> o+2 shift (+2 blocks, stays 4B-aligned)
                U = uu.tile([ROWS, 4, TW - 2], BF16)
                nc.vector.tensor_tensor(
                    out=U[:, :, :],
                    in0=Q[:, 0:4, 0 : TW - 2],
                    in1=Q[:, 4:8, 2:TW],
                    op=MIN,
                )

                # L3: A-half vs B-half
                R = rr.tile([ROWS, 2, TW - 2], BF16)
                nc.vector.tensor_tensor(
                    out=R[:, :, :], in0=U[:, 0:2, :], in1=U[:, 2:4, :], op=MIN
                )

                # L4: out[w] = min(r0[w], r1[w+1])  (odd shift -> 1x; emit f32)
                ot = oo.tile([ROWS, BT], F32)
                nc.vector.tensor_tensor(
                    out=ot[:, :], in0=R[:, 0, 0:BT], in1=R[:, 1, 1 : BT + 1], op=MIN
                )
                nc.sync.dma_start(
                    out=out[:, wbase : wbase + wcnt], in_=ot[:, 0:wcnt]
                )
    nc.finalize()
    return nc


def _host_prep(input_f32, And_weight):
    """Shard + relayout host-side. Returns in_maps for the 8 cores."""
    import ml_dtypes

    xb = np.asarray(input_f32, dtype=np.float32).astype(ml_dtypes.bfloat16)
    # [N, L] -> [N, B, 4] -> [N, 4, B] phase planes
    planes = np.ascontiguousarray(xb.reshape(N, B, S).transpose(0, 2, 1))
    # pad block axis B -> BT*NT + 3 so every tile has its halo
    padded = np.zeros((N, S, NT * BT + 3), dtype=ml_dtypes.bfloat16)
    padded[:, :, :B] = planes
    # [N, 4, padB] -> tiles [N, NT, 4, TW]
    xt = np.empty((N, NT, S, TW), dtype=ml_dtypes.bfloat16)
    for t in range(NT):
        xt[:, t] = padded[:, :, BT * t : BT * t + TW]

    wfull = np.broadcast_to(
        np.asarray(And_weight, dtype=np.float32).reshape(1, K), (ROWS, K)
    ).copy()

    in_maps = []
    for c in range(NCORES):
        in_maps.append(
            {
                "x": np.ascontiguousarray(xt[c * ROWS : (c + 1) * ROWS]),
                "w": wfull,
            }
        )
    return in_maps


def _get_nc():
    if "nc" not in _COMPILED:
        _COMPILED["nc"] = _build_bass()
    return _COMPILED["nc"]


def _run(in_maps, trace=False, **kw):
    from concourse.bass_utils import run_bass_kernel_spmd

    nc = _get_nc()
    res = run_bass_kernel_spmd(
        nc, in_maps, core_ids=list(range(NCORES)), trace=trace, **kw
    )
    return res


def kernel(input, And_weight):
    in_maps = _host_prep(input, And_weight)
    res = _run(in_maps, trace=False)
    out = np.concatenate([res.results[c]["out"] for c in range(NCORES)], axis=0)
    return out.astype(np.float32)
